# revision 47
# baseline (speedup 1.0000x reference)
"""Trainium2 Bass kernel for nn_AttentionBlock (B=4, C=256, H=W=64).

Reference computation:
    q = Wq @ x + bq          [B, 32, N]    (N = H*W = 4096)
    k = Wk @ x + bk          [B, 32, N]
    v = Wv @ x + bv          [B, 256, N]
    scores = q^T k           [B, N, N]
    attn = softmax(scores, axis=-1)
    out = v @ attn^T + x     [B, 256, N]

Sharding: 8 cores = 4 batches x 2 query-row halves (2048 rows each).
Each core computes its (b, half) slab fully independently (no collectives).

Per-core algorithm. Matmul dtypes are chosen per stage: fp16 for the
projections and QK (same PE rate as bf16, enables FWL fast weight load,
2^-11 mantissa keeps the score error ~4e-3), bf16 for PV (the exp weights
span too much dynamic range for fp16/fp8 without max-subtraction).
  - The host reorders x's pixel columns so this core's query half comes
    first; q is then just columns 0:NQ of the shared x tile (softmax sums
    over all keys, so key order is irrelevant as long as v uses the same
    order). x ships as fp16, in 512-column pieces so the first projection
    matmul starts after ~1 us of DMA.
  - k_rep/q_rep: projections with 4x partition replication via the wide
    host-tiled stationary (scoresT contraction dim is d=32, so 4 row-tiled
    QK matmuls on the 128x128 PE array each need their own copy of q/k on
    their 32-partition strip). The PSUM->SBUF round + bias ride the ACT
    engine (Identity activation with per-partition bias AP), which is
    otherwise idle during the projection phase.
  - vT_aug[n, c]: V projection computed directly transposed (stationary =
    x tile, so no on-device transpose), with a ones column appended; the
    PV matmul then produces the softmax denominator as output column 256
    for free. PSUM->SBUF copies alternate ACT (Copy activation) / DVE
    50:50 so neither engine becomes the drain bottleneck.
  - scoresT[j, i] layout means softmax never needs a free-dim reduction:
    exp() is elementwise, the denominator comes from the ones column, and
    no max-subtraction is needed (|scores| < ~40, exp fits fp32/bf16
    range). One 1-term fp16 QK matmul per (jt, strip): operand rounding
    gives ~4e-3 worst-case relative error in the attention weights, far
    inside the 2e-2 gate. Per 4-tile group, 3 exps run on ACT and the
    FIRST tile's exp on DVE via the Schraudolph bit-trick in bf16
    (x * 2^7*log2(e) + magic, convert to int16, bitcast to bf16 ~= exp(x)
    within +-3%); the softmax denominator uses the same approximated
    weights, so the approximation only perturbs the relative weighting.
    Putting the DVE exp FIRST in each group measured ~16 us faster than
    last: the next group's first QK matmul is gated (psA-bank WAR through
    the PE's in-order queue) on that exp, and DVE unblocks it sooner than
    the back of the ACT queue. 3:1 measured best (ACT-only ~18 us slower;
    2:2 and a half-tile split slightly worse).
  - PV: out_unnorm[i, 257] accumulates over j in PSUM (bf16 operands,
    258-wide free dim). PSUM budget: 4 banks of QK scores rotating against
    the exp readers + 4 PV accumulator banks = all 8.
  - normalize+residual is one fused DVE scalar_tensor_tensor:
    y = (1/den) * pv + resid, with resid = (x_q^T + bv) pretransposed on
    the host.

Known wall (measured via A/B For_i-loop deltas, see probe.py): the Tile
framework's per-matmul completion semaphores + the PE's in-order queue
serialize LDWEIGHTS/matmul streams, so the PE runs at ~170 ns per matmul
average; QK row-tile concurrency is bounded by the 4-bank psA WAR against
the exp readers. PSUM capacity (8 banks) blocks deeper buffering. Column-
tiled QK (shared q stream, no replication) measured ~30 us WORSE despite
fewer streamed columns - keep the row-tiled form.
"""

import hashlib
import os
import tempfile

import numpy as np

import concourse.bacc as bacc
import concourse.mybir as mybir
from concourse.tile import TileContext
from concourse.bass_utils import run_bass_kernel_spmd

F32 = mybir.dt.float32
F32R = mybir.dt.float32r
BF16 = mybir.dt.bfloat16
F16 = mybir.dt.float16
I16 = mybir.dt.int16

P = 128          # SBUF partitions
C = 256          # channels
CT = C // P      # 2 channel tiles
N = 4096         # sequence (H*W)
NQ = N // 2      # query rows per core
D = 32           # q/k dim (C/8)
CH = 512         # query-chunk (PSUM free dim)
NCH = NQ // CH   # 4 query chunks
NKCH = N // CH   # 8 key chunks (k projection)
JT = N // P      # 32 key tiles
SUBS = CH // P   # 4 i-subtiles per chunk
NROW = 4         # QK row-tiling factor (128/D)

# Schraudolph fast-exp, bf16 flavor:
#   exp(x) ~= bitcast_bf16(int16(x*EXP_A + EXP_B))
EXP_A = 184.6650322604434         # 2^7 / ln(2)
EXP_B = 16250.496                 # 2^7 * (127 - 0.043)

import ml_dtypes

_NC_CACHE = {}
_NEFF_CACHE = {}
_ONES = np.ones((P, JT, 2), ml_dtypes.bfloat16)


def _precompile(nc):
    """Compile the NEFF at plain-Python level (the in-jax XLA-callback
    compile path deadlocks on large fp32 kernels) and serve it to
    bass2jax's neuronx_cc_hook from a cache keyed on the BIR bytes."""
    from concourse.bass_utils import compile_bass_kernel
    bir = nc.to_json_bytes()
    key = hashlib.sha256(bir).hexdigest()
    if key not in _NEFF_CACHE:
        td = tempfile.mkdtemp(prefix="kneff_")
        neff = compile_bass_kernel(nc, td)
        with open(neff, "rb") as f:
            _NEFF_CACHE[key] = f.read()

    import concourse.bass2jax as b2j
    if not getattr(b2j, "_attn_kernel_neff_patch", False):
        orig = b2j.compile_bir_kernel

        def patched(bir_json, tmpdir, neff_name="file.neff"):
            bj = bir_json if isinstance(bir_json, bytes) else bir_json.encode()
            data = _NEFF_CACHE.get(hashlib.sha256(bj).hexdigest())
            if data is not None:
                p = os.path.join(tmpdir, neff_name)
                with open(p, "wb") as f:
                    f.write(data)
                return p
            return orig(bir_json, tmpdir, neff_name)

        b2j.compile_bir_kernel = patched
        b2j._attn_kernel_neff_patch = True


def _build(iters=1, dve_exp=1, pv_subs=SUBS, probe_io=False, pe_only=False,
           vt_act=2, dve_first=True, exp_split=False, kq_dve_rounds=0):
    nc = bacc.Bacc("TRN2", target_bir_lowering=False)

    # probe_io: big tensors become device-internal so A/B timing probes
    # don't pay (noisy) host<->device transfer for them each call
    big = "Internal" if probe_io else "ExternalInput"
    big_out = "Internal" if probe_io else "ExternalOutput"
    xf = nc.dram_tensor("xf", [P, CT, N], F16, kind=big)
    resid = nc.dram_tensor("resid", [P, NQ // P, C], F32, kind=big)
    wqt = nc.dram_tensor("wqt", [P, CT, P], F16, kind="ExternalInput")
    wkt = nc.dram_tensor("wkt", [P, CT, P], F16, kind="ExternalInput")
    wvt = nc.dram_tensor("wvt", [P, CT, C], F16, kind="ExternalInput")
    bqr = nc.dram_tensor("bqr", [P, 1], F32, kind="ExternalInput")
    bkr = nc.dram_tensor("bkr", [P, 1], F32, kind="ExternalInput")
    vones = nc.dram_tensor("vones", [P, JT, 2], BF16, kind="ExternalInput")
    y = nc.dram_tensor("y", [NQ // P, P, C], F32, kind=big_out)
    if probe_io:
        # tiny external outputs as host-side completion fences
        ydone = nc.dram_tensor("ydone", [1, 4], F32, kind="ExternalOutput")
        ydonek = nc.dram_tensor("ydonek", [1, 8], F16, kind="ExternalOutput")

    ADD = mybir.AluOpType.add
    MUL = mybir.AluOpType.mult
    EXP = mybir.ActivationFunctionType.Exp
    IDENT = mybir.ActivationFunctionType.Identity
    COPY = mybir.ActivationFunctionType.Copy

    from contextlib import ExitStack

    with TileContext(nc) as tc:
        loop_ctx = ExitStack()
        with (
            tc.tile_pool(name="const", bufs=1) as cp,
            tc.tile_pool(name="big", bufs=1) as bp,
            tc.tile_pool(name="expp", bufs=10) as ep,
            tc.tile_pool(name="yp", bufs=5 if probe_io else 4) as yp,
            tc.tile_pool(name="rcpp", bufs=4) as rp,
            tc.tile_pool(name="psA", bufs=4, space="PSUM") as psA,
            tc.tile_pool(name="psPV", bufs=1, space="PSUM") as psPV,
        ):
            if iters > 1:
                loop_ctx.enter_context(tc.For_i(
                    0, iters, 1,
                    hint_engines=(
                        mybir.EngineType.PE,
                        mybir.EngineType.Activation,
                        mybir.EngineType.DVE,
                        mybir.EngineType.SP,
                        mybir.EngineType.Pool,
                    )))
            wq_sb = cp.tile([P, CT, P], F16, tag="wq")
            wk_sb = cp.tile([P, CT, P], F16, tag="wk")
            wv_sb = cp.tile([P, CT, C], F16, tag="wv")
            bq_sb = cp.tile([P, 1], F32, tag="bq")
            bk_sb = cp.tile([P, 1], F32, tag="bk")
            xf_sb = bp.tile([P, CT, N], F16, tag="xf")
            re_sb = bp.tile([P, NQ // P, C], F32, tag="resid")
            # fp16 (not f32r): same PE rate, FWL fast weight load applies,
            # and 2^-11 mantissa keeps the score error ~4e-3
            krh_sb = bp.tile([P, N], F16, tag="krh")
            qrh_sb = bp.tile([P, NQ], F16, tag="qrh")
            vt_sb = bp.tile([P, JT, C + 2], BF16, tag="vt")

            nc.sync.dma_start(out=wq_sb, in_=wqt[:, :, :])
            nc.sync.dma_start(out=wk_sb, in_=wkt[:, :, :])
            nc.sync.dma_start(out=wv_sb, in_=wvt[:, :, :])
            nc.sync.dma_start(out=bq_sb, in_=bqr[:, :])
            nc.sync.dma_start(out=bk_sb, in_=bkr[:, :])
            # xf in key-chunk pieces so the first projection matmuls can
            # start as soon as the first 512 columns land
            for ch in range(NKCH):
                sl = slice(ch * CH, (ch + 1) * CH)
                nc.sync.dma_start(out=xf_sb[:, :, sl], in_=xf[:, :, sl])
            # ones column for the softmax-denominator trick
            nc.sync.dma_start(out=vt_sb[:, :, C:C + 2], in_=vones[:, :, :])
            nc.sync.dma_start(out=re_sb, in_=resid[:, :, :])

            # ---- k_rep / q_rep: projections, replicated on 4 partition
            # strips via PE col-tiling of the wide stationary (host tiles
            # the weights 4x along M), so the PSUM tile ends up [128, ch]
            # = 4 stacked copies of k (or q). PSUM->SBUF round + bias on
            # the ACT engine (idle during this phase).
            for w_sb, b_sb, dsth, nch in (
                (wk_sb, bk_sb, krh_sb, NKCH),
                (wq_sb, bq_sb, qrh_sb, NCH),
            ):
                for ch in range(nch):
                    sl = slice(ch * CH, (ch + 1) * CH)
                    ps = psA.tile([P, CH], F32, tag="ps")
                    for ct in range(CT):
                        nc.tensor.matmul(
                            ps[:, :],
                            w_sb[:, ct, :],
                            xf_sb[:, ct, sl],
                            start=(ct == 0),
                            stop=(ct == CT - 1),
                        )
                    if kq_dve_rounds and ch % kq_dve_rounds == 0:
                        nc.vector.tensor_scalar(
                            out=dsth[:, sl], in0=ps[:, :],
                            scalar1=b_sb[:, :], scalar2=None, op0=ADD)
                    else:
                        nc.scalar.activation(
                            dsth[:, sl], ps[:, :], IDENT, bias=b_sb[:, :])

            # ---- vT_aug projection: vT[n, c] = sum_c' x[c', n] * Wv[c, c']
            # PSUM->SBUF copies split ACT/DVE so the drain keeps pace with
            # the PE (each engine alone is slower than the PE here).
            for jt in range(JT):
                ps = psA.tile([P, C], F32, tag="ps")
                for ct in range(CT):
                    nc.tensor.matmul(
                        ps[:, :],
                        xf_sb[:, ct, jt * P:(jt + 1) * P],
                        wv_sb[:, ct, :],
                        start=(ct == 0),
                        stop=(ct == CT - 1),
                    )
                if jt % vt_act == 0:
                    nc.scalar.activation(vt_sb[:, jt, 0:C], ps[:, :], COPY)
                else:
                    nc.vector.tensor_copy(out=vt_sb[:, jt, 0:C], in_=ps[:, :])

            # ---- attention: per query chunk of 512 rows
            last_yt = None
            for ch in range(NCH):
                # out_unnorm accumulators [i, c+2] for the 4 i-subtiles
                pv = [psPV.tile([P, C + 2], F32, tag=f"pv{s}", name=f"pv{s}")
                      for s in range(SUBS)]
                ets = [None] * NROW

                def qk_group(g):
                    # 4 row-tiled K=32 matmuls on the PE; exp on ACT
                    # (3/group) + DVE Schraudolph (1/group)
                    for r in range(NROW):
                        jt = NROW * g + r
                        rs = slice(32 * r, 32 * (r + 1))
                        js = slice(jt * P, (jt + 1) * P)
                        cs = slice(ch * CH, (ch + 1) * CH)
                        ps = psA.tile([P, CH], F32, tag="ps")
                        nc.tensor.matmul(
                            ps[:, :], krh_sb[rs, js], qrh_sb[rs, cs],
                            start=True, stop=True,
                            tile_position=(32 * r, 0),
                        )
                        if pe_only:
                            # timing probe: no exp; PV reads vt instead
                            ets[r] = None
                            continue
                        et = ep.tile([P, CH], BF16, tag="exp")
                        on_dve = (r < int(dve_exp)) if dve_first else (
                            r >= NROW - int(dve_exp))
                        if on_dve:
                            nc.vector.tensor_scalar(
                                out=et.bitcast(I16)[:, :], in0=ps[:, :],
                                scalar1=EXP_A, scalar2=EXP_B,
                                op0=MUL, op1=ADD)
                        elif exp_split and r == NROW - 1:
                            # half DVE Schraudolph / half ACT exact
                            h = CH // 2
                            nc.vector.tensor_scalar(
                                out=et.bitcast(I16)[:, 0:h], in0=ps[:, 0:h],
                                scalar1=EXP_A, scalar2=EXP_B,
                                op0=MUL, op1=ADD)
                            nc.scalar.activation(
                                et[:, h:CH], ps[:, h:CH], EXP)
                        else:
                            nc.scalar.activation(et[:, :], ps[:, :], EXP)
                        ets[r] = et

                qk_group(0)
                for g in range(JT // NROW):
                    cur = list(ets)
                    if g + 1 < JT // NROW:
                        qk_group(g + 1)
                    for r in range(NROW):
                        jt = NROW * g + r
                        for s in range(pv_subs):
                            lhs = (vt_sb[:, jt, 0:P] if pe_only
                                   else cur[r][:, s * P:(s + 1) * P])
                            nc.tensor.matmul(
                                pv[s][:, :],
                                lhs,
                                vt_sb[:, jt, :],
                                start=(jt == 0),
                                stop=(jt == JT - 1),
                            )

                for s in range(pv_subs):
                    t = ch * SUBS + s
                    rc = rp.tile([P, 1], F32, tag="rc")
                    nc.vector.reciprocal(rc[:, :], pv[s][:, C:C + 1])
                    yt = yp.tile([P, C], F32, tag="yt")
                    # y = (1/den) * pv + resid, fused on DVE
                    nc.vector.scalar_tensor_tensor(
                        out=yt[:, :], in0=pv[s][:, 0:C], scalar=rc[:, :],
                        in1=re_sb[:, t, :], op0=MUL, op1=ADD)
                    nc.sync.dma_start(out=y[t, :, :], in_=yt[:, :])
                    last_yt = yt

            loop_ctx.close()
            if probe_io:
                if last_yt is not None:
                    nc.sync.dma_start(out=ydone[:, :], in_=last_yt[0:1, 0:4])
                else:
                    nc.sync.dma_start(out=ydonek[:, :], in_=krh_sb[0:1, 0:8])
    nc.compile()
    return nc


def _make_in_maps(inputs):
    """Per-core input tensors. The pixel columns of x are rotated so each
    core's query half comes first (key order is irrelevant to attention)."""
    x = np.ascontiguousarray(inputs["x"], np.float32)
    Wq = np.asarray(inputs["Wq"], np.float32)
    bq = np.asarray(inputs["bq"], np.float32)
    Wk = np.asarray(inputs["Wk"], np.float32)
    bk = np.asarray(inputs["bk"], np.float32)
    Wv = np.asarray(inputs["Wv"], np.float32)
    bv = np.asarray(inputs["bv"], np.float32)

    wqt = np.ascontiguousarray(
        np.tile(Wq.T.reshape(CT, P, D).transpose(1, 0, 2), (1, 1, NROW))
        .astype(np.float16))
    wkt = np.ascontiguousarray(
        np.tile(Wk.T.reshape(CT, P, D).transpose(1, 0, 2), (1, 1, NROW))
        .astype(np.float16))
    wvt = np.ascontiguousarray(
        Wv.T.reshape(CT, P, C).transpose(1, 0, 2).astype(np.float16))
    bqr = np.ascontiguousarray(np.tile(bq, NROW).reshape(P, 1).astype(np.float32))
    bkr = np.ascontiguousarray(np.tile(bk, NROW).reshape(P, 1).astype(np.float32))

    in_maps = []
    for core in range(8):
        b, h = divmod(core, 2)
        xb = x[b].reshape(C, N)
        if h:
            xb_re = np.concatenate([xb[:, NQ:], xb[:, :NQ]], axis=1)
        else:
            xb_re = xb
        xf_h = np.ascontiguousarray(
            xb_re.reshape(CT, P, N).transpose(1, 0, 2).astype(np.float16))
        res_h = np.ascontiguousarray(
            (xb[:, h * NQ:(h + 1) * NQ].T + bv[None, :])
            .reshape(NQ // P, P, C).transpose(1, 0, 2))
        in_maps.append({
            "xf": xf_h, "resid": res_h,
            "wqt": wqt, "wkt": wkt, "wvt": wvt,
            "bqr": bqr, "bkr": bkr,
            "vones": _ONES,
        })
    return in_maps


def kernel(x, Wq, bq, Wk, bk, Wv, bv):
    if "main" not in _NC_CACHE:
        _NC_CACHE["main"] = _build()
    nc = _NC_CACHE["main"]
    _precompile(nc)

    inputs = {"x": x, "Wq": Wq, "bq": bq, "Wk": Wk, "bk": bk,
              "Wv": Wv, "bv": bv}
    in_maps = _make_in_maps(inputs)

    res = run_bass_kernel_spmd(nc, in_maps, core_ids=list(range(8)))

    B = np.asarray(x).shape[0]
    out = np.empty((B, C, N), np.float32)
    for core in range(8):
        b, h = divmod(core, 2)
        slab = res.results[core]["y"].reshape(NQ, C)
        out[b, :, h * NQ:(h + 1) * NQ] = slab.T
    return out.reshape(B, C, 64, 64)
